# revision 1
# baseline (speedup 1.0000x reference)
"""GCN 2-layer kernel for 8 TRN2 NeuronCores (Bass/Tile, SPMD).

Strategy:
  - Shard dst nodes across 8 cores (N/8 rows each). Within a core, dst nodes
    are permuted by in-degree so 128-node tiles have near-uniform edge counts
    (minimizes padded gather chunks); host un-permutes the output.
  - Layer support GEMMs (x@W1, h@W2) on the PE per 128-node tile.
  - Sparse aggregation agg[d] = sum_e w_e * s[src_e], per dst tile:
      * edges of the tile packed into 128-edge chunks (edge -> SBUF partition),
      * per chunk, indirect_dma_start gathers the 128 source rows (bf16,
        256B) from the all-gathered support table into SBUF,
      * the per-edge one-hot routing matrix M[e, d] = w_e * (dstloc_e == d)
        is built per tile by one DVE tensor_tensor pair using a host iota row
        broadcast against per-edge dstloc/w columns,
      * PE matmul G.T @ M accumulates each chunk into the tile's PSUM:
        psum[f, d] += sum_e G[e, f] * M[e, d].
  - Two DRAM AllGathers (bf16 support tables) provide cross-core edges.
  - log_softmax epilogue per dst tile (PE transpose + DVE/ACT).
"""

import functools


import numpy as np
import ml_dtypes

NCORES = 8


# ---------------------------------------------------------------- host prep


def _derive(n_nodes):
    nper = n_nodes // NCORES
    assert nper * NCORES == n_nodes
    npad = ((nper + 127) // 128) * 128
    t_tiles = npad // 128
    return nper, npad, t_tiles


def _prep_edges(edge_index, edge_w, n_nodes):
    """Group edges by (dst core, degree-sorted dst tile); pack into chunks.

    Returns per-core index/payload arrays and per-tile chunk counts shared by
    all cores (max over cores), plus the per-core dst permutation.
    """
    nper, npad, T = _derive(n_nodes)
    src = np.asarray(edge_index[0], dtype=np.int64).ravel()
    dst = np.asarray(edge_index[1], dtype=np.int64).ravel()
    w = np.asarray(edge_w, dtype=np.float64).ravel()

    core = dst // nper
    local = dst - core * nper
    glob = core * nper + local  # == dst

    # in-degree per node -> per-core degree-descending permutation
    deg = np.bincount(dst, minlength=n_nodes)
    perm = np.empty((NCORES, nper), dtype=np.int64)  # rank -> local node
    rank_of = np.empty(n_nodes, dtype=np.int64)  # global node -> rank in core
    for c in range(NCORES):
        d = deg[c * nper : (c + 1) * nper]
        p = np.argsort(-d, kind="stable")
        perm[c] = p
        inv = np.empty(nper, dtype=np.int64)
        inv[p] = np.arange(nper)
        rank_of[c * nper : (c + 1) * nper] = inv

    rank = rank_of[glob]
    t = rank >> 7
    dloc = rank & 127
    psrc = (src // nper) * npad + (src % nper)
    psrc2 = (src // nper) * npad + rank_of[src]

    # counts per (core, tile)
    key = core * T + t
    counts = np.bincount(key, minlength=NCORES * T).reshape(NCORES, T)
    ch_t = np.maximum(np.ceil(counts.max(axis=0) / 128).astype(np.int64), 1)
    cap_t = ch_t * 128
    tile_off = np.concatenate([[0], np.cumsum(cap_t)])  # slot offset per tile
    total = int(tile_off[-1])  # padded slots per core

    order = np.argsort(key, kind="stable")
    key_s = key[order]
    starts = np.zeros(NCORES * T, dtype=np.int64)
    np.cumsum(counts.ravel()[:-1], out=starts[1:])
    pos = np.arange(len(src), dtype=np.int64) - starts[key_s]
    core_s = key_s // T
    t_s = key_s - core_s * T
    slot = core_s * total + tile_off[t_s] + pos

    g_all = np.zeros(NCORES * total, dtype=np.int32)
    g2_all = np.zeros(NCORES * total, dtype=np.int32)
    d_all = np.zeros(NCORES * total, dtype=ml_dtypes.bfloat16)
    w_all = np.zeros(NCORES * total, dtype=ml_dtypes.bfloat16)
    g_all[slot] = psrc[order].astype(np.int32)
    g2_all[slot] = psrc2[order].astype(np.int32)
    d_all[slot] = dloc[order].astype(ml_dtypes.bfloat16)
    w_all[slot] = w[order].astype(ml_dtypes.bfloat16)

    # reshape per core: slots of tile t -> [128 partitions x ch_t[t] cols],
    # edge i of tile t at (i % 128, i // 128): i.e. fortran-order reshape.
    def to_cols(a):
        out = np.empty((NCORES, 128, int(ch_t.sum())), a.dtype)
        for c in range(NCORES):
            col = 0
            base = c * total
            for tt in range(T):
                n = int(cap_t[tt])
                blk = a[base + tile_off[tt] : base + tile_off[tt] + n]
                out[c, :, col : col + int(ch_t[tt])] = blk.reshape(
                    int(ch_t[tt]), 128
                ).T
                col += int(ch_t[tt])
        return out

    gidx = to_cols(g_all)  # [NCORES, 128, sum_ch] int32 (node order, layer 1)
    gidx2 = to_cols(g2_all)  # rank order (layer 2)
    dcol = to_cols(d_all)  # bf16
    wcol = to_cols(w_all)  # bf16
    return gidx, gidx2, dcol, wcol, ch_t, perm


# ---------------------------------------------------------------- bass build


@functools.lru_cache(maxsize=4)
def _build(n_nodes, f_in, h, c_out, ch_key):
    import concourse.bacc as bacc
    import concourse.bass as bass
    import concourse.mybir as mybir
    import concourse.tile as tile

    f32 = mybir.dt.float32
    bf16 = mybir.dt.bfloat16
    i32 = mybir.dt.int32
    AF = mybir.ActivationFunctionType
    ALU = mybir.AluOpType
    AX = mybir.AxisListType

    ch_t = list(ch_key)
    nper, npad, T = _derive(n_nodes)
    kf = f_in // 128
    assert f_in % 128 == 0 and h == 128
    sum_ch = sum(ch_t)
    col_off = [0]
    for v in ch_t:
        col_off.append(col_off[-1] + v)

    nc = bacc.Bacc("TRN2", target_bir_lowering=False)

    x_in = nc.dram_tensor("x", [f_in, npad], bf16, kind="ExternalInput")
    gidx_in = nc.dram_tensor("gidx", [128, sum_ch], i32, kind="ExternalInput")
    gidx2_in = nc.dram_tensor("gidx2", [128, sum_ch], i32, kind="ExternalInput")
    dcol_in = nc.dram_tensor("dcol", [128, sum_ch], bf16, kind="ExternalInput")
    wcol_in = nc.dram_tensor("wcol", [128, sum_ch], bf16, kind="ExternalInput")
    iota_in = nc.dram_tensor("iota", [128, 128], bf16, kind="ExternalInput")
    w1_in = nc.dram_tensor("w1", [f_in, h], bf16, kind="ExternalInput")
    w2_in = nc.dram_tensor("w2", [h, c_out], bf16, kind="ExternalInput")
    b1_in = nc.dram_tensor("b1", [h, 1], f32, kind="ExternalInput")
    b2_in = nc.dram_tensor("b2", [c_out, 1], f32, kind="ExternalInput")
    ibf_in = nc.dram_tensor("ibf", [128, 128], bf16, kind="ExternalInput")
    if32_in = nc.dram_tensor("if32", [128, 128], f32, kind="ExternalInput")
    out_t = nc.dram_tensor("out", [npad, c_out], f32, kind="ExternalOutput")

    allcores = [list(range(NCORES))]
    TG = 7  # tiles per idx-load group
    gw_max = max(
        col_off[min(g0 + TG, T)] - col_off[g0] for g0 in range(0, T, TG)
    )

    with tile.TileContext(nc) as tc:
        with (
            tc.tile_pool(name="consts", bufs=1) as consts,
            tc.tile_pool(name="dram", bufs=1, space="DRAM") as dram,
        ):
            w1_sb = consts.tile([128, kf, 128], bf16, name="w1_sb")
            for k in range(kf):
                nc.sync.dma_start(w1_sb[:, k, :], w1_in[k * 128 : (k + 1) * 128, :])
            w2_sb = consts.tile([128, c_out], bf16, name="w2_sb")
            nc.sync.dma_start(w2_sb[:], w2_in[:])
            b1_sb = consts.tile([128, 1], f32, name="b1_sb")
            nc.sync.dma_start(b1_sb[:], b1_in[:])
            b2_sb = consts.tile([128, 1], f32, name="b2_sb")
            nc.sync.dma_start(b2_sb[0:c_out, :], b2_in[:])
            iota_sb = consts.tile([128, 128], bf16, name="iota_sb")
            nc.sync.dma_start(iota_sb[:], iota_in[:])
            ibf_sb = consts.tile([128, 128], bf16, name="ibf_sb")
            nc.sync.dma_start(ibf_sb[:], ibf_in[:])
            if32_sb = consts.tile([128, 128], f32, name="if32_sb")
            nc.sync.dma_start(if32_sb[:], if32_in[:])

            s1_loc = dram.tile([npad, h], bf16, name="s1_loc")
            s1_full = dram.tile([NCORES * npad, h], bf16, name="s1_full",
                                addr_space="Shared")
            s2_loc = dram.tile([npad, c_out], bf16, name="s2_loc")
            s2_full = dram.tile([NCORES * npad, c_out], bf16, name="s2_full",
                                addr_space="Shared")

            # ---- phase 1: s1 = x @ W1 (bf16 rows into s1_loc)
            with (
                tc.tile_pool(name="p1", bufs=3) as p1,
                tc.tile_pool(name="p1ps", bufs=2, space="PSUM") as p1ps,
            ):
                for t in range(T):
                    ps = p1ps.tile([128, 128], f32, name="s1ps", tag="s1ps")
                    for k in range(kf):
                        xT = p1.tile([128, 128], bf16, name="xT", tag="xT")
                        nc.sync.dma_start(
                            xT[:],
                            x_in[k * 128 : (k + 1) * 128, t * 128 : (t + 1) * 128],
                        )
                        nc.tensor.matmul(
                            ps[:], xT[:], w1_sb[:, k, :],
                            start=(k == 0), stop=(k == kf - 1),
                        )
                    s1sb = p1.tile([128, 128], bf16, name="s1sb", tag="s1sb")
                    nc.scalar.activation(s1sb[:], ps[:], AF.Copy)
                    nc.sync.dma_start(s1_loc[t * 128 : (t + 1) * 128, :], s1sb[:])

            # ---- phase 2: all-gather s1
            nc.gpsimd.collective_compute(
                "AllGather", ALU.bypass, allcores, ins=[s1_loc[:]], outs=[s1_full[:]]
            )

            def agg_layer(src_full, layer):
                idx_src = gidx_in if layer == 1 else gidx2_in
                with (
                    tc.tile_pool(name=f"pidx{layer}", bufs=3) as pidx,
                    tc.tile_pool(name=f"pg{layer}", bufs=24) as pg,
                    tc.tile_pool(name=f"pm{layer}", bufs=4) as pm,
                    tc.tile_pool(name=f"pe{layer}", bufs=3) as pe,
                    tc.tile_pool(name=f"psA{layer}", bufs=4, space="PSUM") as psA,
                    tc.tile_pool(name=f"psB{layer}", bufs=2, space="PSUM") as psB,
                    tc.tile_pool(name=f"psC{layer}", bufs=2, space="PSUM") as psC,
                ):
                    for g0 in range(0, T, TG):
                        gn = min(TG, T - g0)
                        c0, c1 = col_off[g0], col_off[g0 + gn]
                        gcw = c1 - c0
                        gi = pidx.tile([128, gcw], i32, name="gi", tag="gi",
                                       padded_shape=[128, gw_max])
                        nc.sync.dma_start(gi[:], idx_src[:, c0:c1])
                        dc = pidx.tile([128, gcw], bf16, name="dc", tag="dc",
                                       padded_shape=[128, gw_max])
                        nc.sync.dma_start(dc[:], dcol_in[:, c0:c1])
                        wc = pidx.tile([128, gcw], bf16, name="wc", tag="wc",
                                       padded_shape=[128, gw_max])
                        nc.sync.dma_start(wc[:], wcol_in[:, c0:c1])

                        for tt in range(gn):
                            t = g0 + tt
                            ch = ch_t[t]
                            lo = col_off[t] - c0
                            # one-hot routing matrix for the whole tile
                            mb = pm.tile([128, ch, 128], bf16, name="mb", tag="mb",
                                         padded_shape=[128, max(ch_t), 128])
                            nc.vector.tensor_tensor(
                                out=mb[:],
                                in0=dc[:, lo : lo + ch][:, :, None].broadcast_to(
                                    [128, ch, 128]
                                ),
                                in1=iota_sb[:][:, None, :].broadcast_to(
                                    [128, ch, 128]
                                ),
                                op=mybir.AluOpType.is_equal,
                            )
                            nc.vector.tensor_tensor(
                                out=mb[:],
                                in0=mb[:],
                                in1=wc[:, lo : lo + ch][:, :, None].broadcast_to(
                                    [128, ch, 128]
                                ),
                                op=mybir.AluOpType.mult,
                            )
                            gw = 128 if layer == 1 else c_out
                            ps = psA.tile([gw, 128], f32, name="aggps", tag="aggps",
                                          padded_shape=[128, 128])
                            for cc in range(ch):
                                gb = pg.tile([128, gw], bf16, name="gb", tag="gb",
                                             padded_shape=[128, 128])
                                nc.gpsimd.indirect_dma_start(
                                    out=gb[:], out_offset=None, in_=src_full[:],
                                    in_offset=bass.IndirectOffsetOnAxis(
                                        ap=gi[:, lo + cc : lo + cc + 1], axis=0
                                    ),
                                )
                                nc.tensor.matmul(
                                    ps[:], gb[:], mb[:, cc, :],
                                    start=(cc == 0), stop=(cc == ch - 1),
                                )
                            if layer == 1:
                                h_sb = pe.tile([128, 128], bf16, name="h_sb", tag="h")
                                nc.scalar.activation(
                                    h_sb[:], ps[:], AF.Relu, bias=b1_sb[:]
                                )
                                ps2 = psB.tile([c_out, 128], f32, name="s2ps", tag="s2ps")
                                nc.tensor.matmul(
                                    ps2[:], w2_sb[:], h_sb[:], start=True, stop=True
                                )
                                s2c = pe.tile([c_out, 128], bf16, name="s2c", tag="s2c")
                                nc.scalar.activation(s2c[:], ps2[:], AF.Copy)
                                psT = psC.tile([128, c_out], bf16, name="s2T", tag="s2T")
                                nc.tensor.transpose(
                                    psT[:], s2c[:], ibf_sb[0:c_out, 0:c_out]
                                )
                                s2row = pe.tile([128, c_out], bf16, name="s2row",
                                                tag="s2r")
                                nc.scalar.activation(s2row[:], psT[:], AF.Copy)
                                nc.sync.dma_start(
                                    s2_loc[t * 128 : (t + 1) * 128, :], s2row[:]
                                )
                            else:
                                sb2 = pe.tile([c_out, 128], f32, name="sb2", tag="sb2")
                                nc.vector.tensor_scalar_add(
                                    sb2[:], ps[:], b2_sb[0:c_out, :]
                                )
                                psO = psB.tile([128, c_out], f32, name="psO", tag="psO")
                                nc.tensor.transpose(
                                    psO[:], sb2[:], if32_sb[0:c_out, 0:c_out]
                                )
                                mx = pe.tile([128, 1], f32, name="mx", tag="mx")
                                nc.vector.tensor_reduce(
                                    mx[:], psO[:], axis=AX.X, op=ALU.max, negate=True
                                )
                                e_sb = pe.tile([128, c_out], f32, name="e_sb", tag="esb")
                                ssum = pe.tile([128, 1], f32, name="ssum", tag="ssum")
                                nc.scalar.activation(
                                    e_sb[:], psO[:], AF.Exp, bias=mx[:],
                                    accum_out=ssum[:],
                                )
                                lns = pe.tile([128, 1], f32, name="lns", tag="lns")
                                nc.scalar.activation(lns[:], ssum[:], AF.Ln)
                                off = pe.tile([128, 1], f32, name="off", tag="off")
                                nc.vector.tensor_sub(off[:], mx[:], lns[:])
                                o_sb = pe.tile([128, c_out], f32, name="o_sb", tag="osb")
                                nc.vector.tensor_scalar_add(o_sb[:], psO[:], off[:])
                                nc.sync.dma_start(
                                    out_t[t * 128 : (t + 1) * 128, :], o_sb[:]
                                )

            agg_layer(s1_full, 1)
            nc.gpsimd.collective_compute(
                "AllGather", ALU.bypass, allcores, ins=[s2_loc[:]], outs=[s2_full[:]]
            )
            agg_layer(s2_full, 2)

    nc.compile()
    return nc


# ---------------------------------------------------------------- entry


def _run(x, edge_index, edge_w, W1, b1, W2, b2, trace=False):
    from concourse.bass_utils import run_bass_kernel_spmd

    x = np.asarray(x)
    n_nodes, f_in = x.shape
    h = W1.shape[1]
    c_out = W2.shape[1]
    nper, npad, T = _derive(n_nodes)

    gidx, gidx2, dcol, wcol, ch_t, perm = _prep_edges(edge_index, edge_w, n_nodes)

    w1b = np.asarray(W1, dtype=ml_dtypes.bfloat16)
    w2b = np.asarray(W2, dtype=ml_dtypes.bfloat16)
    b1c = np.asarray(b1, dtype=np.float32).reshape(h, 1)
    b2c = np.asarray(b2, dtype=np.float32).reshape(c_out, 1)
    iota = np.tile(np.arange(128, dtype=ml_dtypes.bfloat16)[None, :], (128, 1))
    ibf = np.eye(128, dtype=ml_dtypes.bfloat16)
    if32 = np.eye(128, dtype=np.float32)

    xb = np.zeros((NCORES, f_in, npad), dtype=ml_dtypes.bfloat16)
    xr = np.asarray(x, dtype=ml_dtypes.bfloat16).reshape(NCORES, nper, f_in)
    xb[:, :, :nper] = xr.transpose(0, 2, 1)

    in_maps = []
    for c in range(NCORES):
        in_maps.append(
            {"x": xb[c], "gidx": gidx[c], "gidx2": gidx2[c],
             "dcol": dcol[c], "wcol": wcol[c],
             "iota": iota, "w1": w1b, "w2": w2b, "b1": b1c, "b2": b2c,
             "ibf": ibf, "if32": if32}
        )

    nc = _build(n_nodes, f_in, h, c_out, tuple(int(v) for v in ch_t))
    res = run_bass_kernel_spmd(nc, in_maps, list(range(NCORES)), trace=trace)

    out = np.empty((n_nodes, c_out), dtype=np.float32)
    for c in range(NCORES):
        ranked = res.results[c]["out"]  # [npad, c_out], rank r = node perm[c][r]
        inv = np.empty(nper, dtype=np.int64)
        inv[perm[c]] = np.arange(nper)
        out[c * nper : (c + 1) * nper] = ranked[inv]
    return out, res


def kernel(x, edge_index, edge_w, W1, b1, W2, b2):
    out, _ = _run(x, edge_index, edge_w, W1, b1, W2, b2)
    return out



# revision 9
# speedup vs baseline: 24.9923x; 24.9923x over previous
"""GCN 2-layer kernel for 8 TRN2 NeuronCores (Bass/Tile, SPMD).

Strategy:
  - Shard dst nodes across 8 cores (N/8 rows each), degree-sorted so 128-node
    tiles have near-uniform edge counts; host un-permutes the output. All
    tables (x, s1, h) are kept in rank order so one index tensor serves both
    layers.
  - Layer algebra: h = relu(A @ (x@W1) + b1); out = logsoftmax((A@h) @ W2 + b2)
    (aggregation commutes with the dense GEMM, so layer 2 aggregates h at
    128 features = 256B rows, the dma_gather granularity).
  - Sparse aggregation per dst tile via PE: per 128-edge chunk,
    psum[f, d] += G[e, f]^T * M[e, d], where G holds gathered source rows and
    M[e, d] = w_e * (dstloc_e == d) is built by two DVE broadcasts.
  - Source rows are fetched with big dma_gather ops (amortized SWDGE cost:
    ~1us + 0.34ns/row) instead of per-chunk indirect DMAs. int16 index range
    is handled by splitting the all-gathered table into 4 blocks of 2*npad
    rows; edges are grouped (tile-group, block) and sorted by source row for
    HBM locality.
  - Two DRAM AllGathers (bf16 s1 and h tables) provide cross-core edges.
"""

import functools

import numpy as np
import ml_dtypes

NCORES = 8
TG = 4  # tiles per gather group
NBLK = 4  # index blocks (2 cores per block; 2*npad rows < 32768)
OPC = 8  # max gather cols per dma_gather op (1024 idx = SWDGE ring capacity)
NQ = 4  # SWDGE queues (round-robin)


def _derive(n_nodes):
    nper = n_nodes // NCORES
    assert nper * NCORES == n_nodes
    npad = ((nper + 127) // 128) * 128
    t_tiles = npad // 128
    return nper, npad, t_tiles


def _layout(ch_tq, T):
    """Column layouts shared by host packing and kernel build.

    mb/dcol/wcol: tile-major (tile t, then block q, then chunk).
    gather buffer: per group of TG tiles, block-major (q, then t, then chunk)
    so each (group, q) dma_gather writes one contiguous column range.
    """
    ch_t = ch_tq.sum(1)
    mb_off = np.zeros(T + 1, np.int64)
    mb_off[1:] = np.cumsum(ch_t)
    mq_off = np.zeros((T, 4), np.int64)
    mq_off[:, 1:] = np.cumsum(ch_tq, axis=1)[:, :-1]
    groups = [list(range(g, min(g + TG, T))) for g in range(0, T, TG)]
    seg_off = np.zeros((T, 4), np.int64)
    op_cols, gq_off, gb_off, gcw = [], [], [], []
    gbase = 0
    for g in groups:
        oc = [int(ch_tq[g, qq].sum()) for qq in range(4)]
        qb = [0, oc[0], oc[0] + oc[1], oc[0] + oc[1] + oc[2]]
        for qq in range(4):
            off = 0
            for tt in g:
                seg_off[tt, qq] = off
                off += int(ch_tq[tt, qq])
        op_cols.append(oc)
        gq_off.append(qb)
        gcw.append(sum(oc))
        gb_off.append(gbase)
        gbase += sum(oc)
    return ch_t, mb_off, mq_off, groups, seg_off, op_cols, gq_off, gb_off, gcw


def _prep_edges(edge_index, edge_w, n_nodes):
    nper, npad, T = _derive(n_nodes)
    bs = 2 * npad
    src = np.asarray(edge_index[0], dtype=np.int64).ravel()
    dst = np.asarray(edge_index[1], dtype=np.int64).ravel()
    w = np.asarray(edge_w, dtype=np.float64).ravel()

    deg = np.bincount(dst, minlength=n_nodes)
    perm = np.empty((NCORES, nper), dtype=np.int64)  # rank -> local node
    rank_of = np.empty(n_nodes, dtype=np.int64)
    for c in range(NCORES):
        d = deg[c * nper : (c + 1) * nper]
        p = np.argsort(-d, kind="stable")
        perm[c] = p
        inv = np.empty(nper, dtype=np.int64)
        inv[p] = np.arange(nper)
        rank_of[c * nper : (c + 1) * nper] = inv

    trow = (src // nper) * npad + rank_of[src]  # rank-order table row
    q = (src // nper) >> 1
    rel = (trow - q * bs).astype(np.int64)  # < 2*npad
    core = dst // nper
    rank = rank_of[dst]
    t = rank >> 7
    dloc = rank & 127

    key = ((core * T + t) << 2) | q
    counts = np.bincount(key, minlength=NCORES * T * 4).reshape(NCORES, T, 4)
    ch_tq = np.ceil(counts.max(axis=0) / 128).astype(np.int64)  # shared [T,4]
    fix = ch_tq.sum(1) == 0
    ch_tq[fix, 0] = 1

    (ch_t, mb_off, mq_off, groups, seg_off, op_cols, gq_off, gb_off, gcw) = (
        _layout(ch_tq, T)
    )
    sum_ch = int(ch_t.sum())

    # per-(t,q): gather-linear and mb-linear base slots
    grp_of = np.arange(T) // TG
    gcolbase = np.empty((T, 4), np.int64)
    for tt in range(T):
        gi_ = grp_of[tt]
        for qq in range(4):
            gcolbase[tt, qq] = gb_off[gi_] + gq_off[gi_][qq] + seg_off[tt, qq]
    mcolbase = mb_off[:T, None] + mq_off

    # slot assignment: edges of (core,t,q) sorted by rel for HBM locality
    order = np.lexsort((rel, key))
    key_s = key[order]
    counts_flat = counts.reshape(-1)
    starts = np.zeros(NCORES * T * 4, dtype=np.int64)
    np.cumsum(counts_flat[:-1], out=starts[1:])
    pos = np.arange(len(src), dtype=np.int64) - starts[key_s]
    kt = key_s >> 2
    core_s = kt // T
    t_s = kt - core_s * T
    q_s = key_s & 3

    SC = sum_ch * 128
    L = gcolbase[t_s, q_s] * 128 + pos  # gather-linear slot within core
    M = mcolbase[t_s, q_s] * 128 + pos  # mb-linear slot within core

    idxlin = np.zeros(NCORES * SC, dtype=np.int16)
    dlin = np.zeros(NCORES * SC, dtype=ml_dtypes.bfloat16)
    wlin = np.zeros(NCORES * SC, dtype=ml_dtypes.bfloat16)
    idxlin[core_s * SC + L] = rel[order].astype(np.int16)
    dlin[core_s * SC + M] = dloc[order].astype(ml_dtypes.bfloat16)
    wlin[core_s * SC + M] = w[order].astype(ml_dtypes.bfloat16)

    # gidx16[c, p, s] = idxlin[c, s*16 + p%16]  (16-partition interleave,
    # valid because every op base is 128-aligned)
    A = idxlin.reshape(NCORES, SC // 16, 16)
    gidx16 = np.ascontiguousarray(
        np.tile(A.transpose(0, 2, 1), (1, 8, 1))
    )  # [NCORES, 128, sum_ch*8]
    dcol = np.ascontiguousarray(
        dlin.reshape(NCORES, sum_ch, 128).transpose(0, 2, 1)
    )
    wcol = np.ascontiguousarray(
        wlin.reshape(NCORES, sum_ch, 128).transpose(0, 2, 1)
    )
    return gidx16, dcol, wcol, ch_tq, perm


# ---------------------------------------------------------------- bass build


@functools.lru_cache(maxsize=4)
def _build(n_nodes, f_in, h, c_out, ch_key):
    import concourse.bacc as bacc
    import concourse.bass as bass
    import concourse.mybir as mybir
    import concourse.tile as tile

    f32 = mybir.dt.float32
    bf16 = mybir.dt.bfloat16
    i16 = mybir.dt.int16
    AF = mybir.ActivationFunctionType
    ALU = mybir.AluOpType
    AX = mybir.AxisListType

    nper, npad, T = _derive(n_nodes)
    bs = 2 * npad
    kf = f_in // 128
    assert f_in % 128 == 0 and h == 128
    ch_tq = np.array(ch_key, dtype=np.int64).reshape(T, 4)
    (ch_t, mb_off, mq_off, groups, seg_off, op_cols, gq_off, gb_off, gcw) = (
        _layout(ch_tq, T)
    )
    sum_ch = int(ch_t.sum())
    gw_max = max(gcw)
    cht_max = int(ch_t.max())
    mw_max = max(
        int(mb_off[g[-1] + 1] - mb_off[g[0]]) for g in groups
    )

    nc = bacc.Bacc("TRN2", target_bir_lowering=False, num_swdge_queues=NQ)

    x_in = nc.dram_tensor("x", [f_in, npad], bf16, kind="ExternalInput")
    gidx_in = nc.dram_tensor("gidx", [128, sum_ch * 8], i16, kind="ExternalInput")
    dcol_in = nc.dram_tensor("dcol", [128, sum_ch], bf16, kind="ExternalInput")
    wcol_in = nc.dram_tensor("wcol", [128, sum_ch], bf16, kind="ExternalInput")
    iota_in = nc.dram_tensor("iota", [128, 128], bf16, kind="ExternalInput")
    w1_in = nc.dram_tensor("w1", [f_in, h], bf16, kind="ExternalInput")
    w2_in = nc.dram_tensor("w2", [h, c_out], bf16, kind="ExternalInput")
    b1_in = nc.dram_tensor("b1", [h, 1], f32, kind="ExternalInput")
    b2bc_in = nc.dram_tensor("b2bc", [128, c_out], f32, kind="ExternalInput")
    ibf_in = nc.dram_tensor("ibf", [128, 128], bf16, kind="ExternalInput")
    out_t = nc.dram_tensor("out", [npad, c_out], f32, kind="ExternalOutput")

    allcores = [list(range(NCORES))]

    with tile.TileContext(nc) as tc:
        with (
            tc.tile_pool(name="consts", bufs=1) as consts,
            tc.tile_pool(name="dram", bufs=1, space="DRAM") as dram,
        ):
            w1_sb = consts.tile([128, kf, 128], bf16, name="w1_sb")
            for k in range(kf):
                nc.sync.dma_start(w1_sb[:, k, :], w1_in[k * 128 : (k + 1) * 128, :])
            w2_sb = consts.tile([128, c_out], bf16, name="w2_sb")
            nc.sync.dma_start(w2_sb[:], w2_in[:])
            b1_sb = consts.tile([128, 1], f32, name="b1_sb")
            nc.sync.dma_start(b1_sb[:], b1_in[:])
            b2bc_sb = consts.tile([128, c_out], f32, name="b2bc_sb")
            nc.sync.dma_start(b2bc_sb[:], b2bc_in[:])
            iota_sb = consts.tile([128, 128], bf16, name="iota_sb")
            nc.sync.dma_start(iota_sb[:], iota_in[:])
            ibf_sb = consts.tile([128, 128], bf16, name="ibf_sb")
            nc.sync.dma_start(ibf_sb[:], ibf_in[:])

            s1_loc = dram.tile([npad, h], bf16, name="s1_loc")
            s1_full = dram.tile([NCORES * npad, h], bf16, name="s1_full",
                                addr_space="Shared")
            h_loc = dram.tile([npad, h], bf16, name="h_loc")
            h_full = dram.tile([NCORES * npad, h], bf16, name="h_full",
                               addr_space="Shared")

            # ---- phase 1: s1 = x @ W1 (rank-ordered rows)
            with (
                tc.tile_pool(name="p1", bufs=3) as p1,
                tc.tile_pool(name="p1ps", bufs=2, space="PSUM") as p1ps,
            ):
                for t in range(T):
                    ps = p1ps.tile([128, 128], f32, name="s1ps", tag="s1ps")
                    for k in range(kf):
                        xT = p1.tile([128, 128], bf16, name="xT", tag="xT")
                        nc.sync.dma_start(
                            xT[:],
                            x_in[k * 128 : (k + 1) * 128, t * 128 : (t + 1) * 128],
                        )
                        nc.tensor.matmul(
                            ps[:], xT[:], w1_sb[:, k, :],
                            start=(k == 0), stop=(k == kf - 1),
                        )
                    s1sb = p1.tile([128, 128], bf16, name="s1sb", tag="s1sb")
                    nc.scalar.activation(s1sb[:], ps[:], AF.Copy)
                    nc.sync.dma_start(s1_loc[t * 128 : (t + 1) * 128, :], s1sb[:])

            nc.gpsimd.collective_compute(
                "AllGather", ALU.bypass, allcores, ins=[s1_loc[:]], outs=[s1_full[:]]
            )

            def agg_layer(src_full, layer):
                self_rr = [0]  # round-robin queue counter
                with (
                    tc.tile_pool(name=f"pidx{layer}", bufs=2) as pidx,
                    tc.tile_pool(name=f"pdcw{layer}", bufs=2) as pdcw,
                    tc.tile_pool(name=f"pg{layer}", bufs=2) as pg,
                    tc.tile_pool(name=f"pm{layer}", bufs=3) as pm,
                    tc.tile_pool(name=f"pe{layer}", bufs=3) as pe,
                    tc.tile_pool(name=f"psA{layer}", bufs=3, space="PSUM") as psA,
                    tc.tile_pool(name=f"psB{layer}", bufs=2, space="PSUM") as psB,
                ):
                    for gi_, g in enumerate(groups):
                        gw = gcw[gi_]
                        c0 = gb_off[gi_]
                        m0 = int(mb_off[g[0]])
                        m1 = int(mb_off[g[-1] + 1])
                        mcw = m1 - m0
                        ix = pidx.tile([128, gw * 8], i16, name="ix", tag="ix",
                                       padded_shape=[128, gw_max * 8])
                        nc.sync.dma_start(ix[:], gidx_in[:, c0 * 8 : (c0 + gw) * 8])
                        dc = pdcw.tile([128, mcw], bf16, name="dc", tag="dc",
                                       padded_shape=[128, mw_max])
                        nc.sync.dma_start(dc[:], dcol_in[:, m0:m1])
                        wc = pdcw.tile([128, mcw], bf16, name="wc", tag="wc",
                                       padded_shape=[128, mw_max])
                        nc.sync.dma_start(wc[:], wcol_in[:, m0:m1])

                        gbuf = pg.tile([128, gw, 128], bf16, name="gbuf", tag="gbuf",
                                       padded_shape=[128, gw_max, 128])
                        for qq in range(4):
                            oc = op_cols[gi_][qq]
                            if oc == 0:
                                continue
                            qb = gq_off[gi_][qq]
                            for a in range(0, oc, OPC):
                                o = min(OPC, oc - a)
                                nc.gpsimd.dma_gather(
                                    gbuf[:, qb + a : qb + a + o, :],
                                    src_full[qq * bs : (qq + 1) * bs, :],
                                    ix[:, (qb + a) * 8 : (qb + a + o) * 8],
                                    num_idxs=o * 128,
                                    num_idxs_reg=o * 128,
                                    elem_size=128,
                                    queue_num=self_rr[0] % NQ,
                                )
                                self_rr[0] += 1

                        for t in g:
                            cht = int(ch_t[t])
                            mloc = int(mb_off[t]) - m0
                            mb = pm.tile([128, cht, 128], bf16, name="mb", tag="mb",
                                         padded_shape=[128, cht_max, 128])
                            nc.vector.tensor_tensor(
                                out=mb[:],
                                in0=dc[:, mloc : mloc + cht][:, :, None].broadcast_to(
                                    [128, cht, 128]
                                ),
                                in1=iota_sb[:][:, None, :].broadcast_to(
                                    [128, cht, 128]
                                ),
                                op=ALU.is_equal,
                            )
                            nc.vector.tensor_tensor(
                                out=mb[:],
                                in0=mb[:],
                                in1=wc[:, mloc : mloc + cht][:, :, None].broadcast_to(
                                    [128, cht, 128]
                                ),
                                op=ALU.mult,
                            )
                            ps = psA.tile([128, 128], f32, name="aggps", tag="aggps")
                            cc = 0
                            for qq in range(4):
                                gqb = gq_off[gi_][qq] + int(seg_off[t, qq])
                                for k in range(int(ch_tq[t, qq])):
                                    nc.tensor.matmul(
                                        ps[:], gbuf[:, gqb + k, :], mb[:, cc, :],
                                        start=(cc == 0), stop=(cc == cht - 1),
                                    )
                                    cc += 1
                            if layer == 1:
                                h_sb = pe.tile([128, 128], bf16, name="h_sb", tag="h")
                                nc.scalar.activation(
                                    h_sb[:], ps[:], AF.Relu, bias=b1_sb[:]
                                )
                                psT = psB.tile([128, 128], bf16, name="hT", tag="hT")
                                nc.tensor.transpose(psT[:], h_sb[:], ibf_sb[:])
                                h_row = pe.tile([128, 128], bf16, name="h_row",
                                                tag="hr")
                                nc.scalar.activation(h_row[:], psT[:], AF.Copy)
                                nc.sync.dma_start(
                                    h_loc[t * 128 : (t + 1) * 128, :], h_row[:]
                                )
                            else:
                                hagg = pe.tile([128, 128], bf16, name="hagg",
                                               tag="hagg")
                                nc.scalar.activation(hagg[:], ps[:], AF.Copy)
                                psO = psB.tile([128, c_out], f32, name="psO",
                                               tag="psO")
                                nc.tensor.matmul(
                                    psO[:], hagg[:], w2_sb[:], start=True, stop=True
                                )
                                z = pe.tile([128, c_out], f32, name="z", tag="z")
                                nc.vector.tensor_tensor(
                                    out=z[:], in0=psO[:], in1=b2bc_sb[:], op=ALU.add
                                )
                                mx = pe.tile([128, 1], f32, name="mx", tag="mx")
                                nc.vector.tensor_reduce(
                                    mx[:], z[:], axis=AX.X, op=ALU.max, negate=True
                                )
                                e_sb = pe.tile([128, c_out], f32, name="e_sb",
                                               tag="esb")
                                ssum = pe.tile([128, 1], f32, name="ssum", tag="ssum")
                                nc.scalar.activation(
                                    e_sb[:], z[:], AF.Exp, bias=mx[:],
                                    accum_out=ssum[:],
                                )
                                lns = pe.tile([128, 1], f32, name="lns", tag="lns")
                                nc.scalar.activation(lns[:], ssum[:], AF.Ln)
                                off = pe.tile([128, 1], f32, name="off", tag="off")
                                nc.vector.tensor_sub(off[:], mx[:], lns[:])
                                o_sb = pe.tile([128, c_out], f32, name="o_sb",
                                               tag="osb")
                                nc.vector.tensor_scalar_add(o_sb[:], z[:], off[:])
                                nc.sync.dma_start(
                                    out_t[t * 128 : (t + 1) * 128, :], o_sb[:]
                                )

            agg_layer(s1_full, 1)
            nc.gpsimd.collective_compute(
                "AllGather", ALU.bypass, allcores, ins=[h_loc[:]], outs=[h_full[:]]
            )
            agg_layer(h_full, 2)

    nc.compile()
    return nc


# ---------------------------------------------------------------- entry


def _prepare(x, edge_index, edge_w, W1, b1, W2, b2):
    x = np.asarray(x)
    n_nodes, f_in = x.shape
    h = W1.shape[1]
    c_out = W2.shape[1]
    nper, npad, T = _derive(n_nodes)

    gidx16, dcol, wcol, ch_tq, perm = _prep_edges(edge_index, edge_w, n_nodes)

    w1b = np.asarray(W1, dtype=ml_dtypes.bfloat16)
    w2b = np.asarray(W2, dtype=ml_dtypes.bfloat16)
    b1c = np.asarray(b1, dtype=np.float32).reshape(h, 1)
    b2bc = np.tile(
        np.asarray(b2, dtype=np.float32).reshape(1, c_out), (128, 1)
    )
    iota = np.tile(np.arange(128, dtype=ml_dtypes.bfloat16)[None, :], (128, 1))
    ibf = np.eye(128, dtype=ml_dtypes.bfloat16)

    # x columns in rank order per core
    xb = np.zeros((NCORES, f_in, npad), dtype=ml_dtypes.bfloat16)
    xr = np.asarray(x, dtype=ml_dtypes.bfloat16).reshape(NCORES, nper, f_in)
    for c in range(NCORES):
        xb[c, :, :nper] = xr[c, perm[c], :].T

    in_maps = []
    for c in range(NCORES):
        in_maps.append(
            {"x": xb[c], "gidx": gidx16[c], "dcol": dcol[c], "wcol": wcol[c],
             "iota": iota, "w1": w1b, "w2": w2b, "b1": b1c, "b2bc": b2bc,
             "ibf": ibf}
        )

    nc = _build(n_nodes, f_in, h, c_out, tuple(int(v) for v in ch_tq.ravel()))
    return nc, in_maps, perm


def _run(x, edge_index, edge_w, W1, b1, W2, b2, trace=False):
    from concourse.bass_utils import run_bass_kernel_spmd

    n_nodes = np.asarray(x).shape[0]
    c_out = W2.shape[1]
    nper, npad, T = _derive(n_nodes)
    nc, in_maps, perm = _prepare(x, edge_index, edge_w, W1, b1, W2, b2)
    res = run_bass_kernel_spmd(nc, in_maps, list(range(NCORES)), trace=trace)

    out = np.empty((n_nodes, c_out), dtype=np.float32)
    for c in range(NCORES):
        ranked = res.results[c]["out"]  # [npad, c_out], rank r = node perm[c][r]
        inv = np.empty(nper, dtype=np.int64)
        inv[perm[c]] = np.arange(nper)
        out[c * nper : (c + 1) * nper] = ranked[inv]
    return out, res


def kernel(x, edge_index, edge_w, W1, b1, W2, b2):
    out, _ = _run(x, edge_index, edge_w, W1, b1, W2, b2)
    return out
